# revision 3
# baseline (speedup 1.0000x reference)
"""BRITS-style RNN imputation kernel for Trainium2 (8 NeuronCores, data-parallel).

v2: restructured from the v1 baseline to shorten the per-step critical path.
  - gamma_h (egam) precomputed in bulk for all T (it only depends on deltas).
  - Imputation algebra refactored so the in-loop chain is
        cc = R + P*xh + Q*(Wf @ ((P+Q)*xh))
    with Q=(1-m)*alpha, P=(1-m)*(1-alpha), R=v+Q*(Wf@v+b_f) all bulk-built.
  - Output stored feature-major bf16; bulk PE-transposes + PSUM->HBM DMAs
    at the end (no per-step transpose/DMA).
  - LSTM tail: chunked tanh (f,i,g,o), A on Pool in parallel with Bt on DVE,
    transpose Cst and tanh_o instead of hh, gamma-multiply fused into evac.
  - Gates for the last step are skipped (h(T) is never used).

Model dims (hardcoded from the problem spec):
  B=256, T=256, C=64, H=512. Per-core batch shard Bl=32.
"""

import os
import sys

sys.path.insert(0, "/opt/trn_rl_repo")

import numpy as np
import ml_dtypes

B, T, C, H = 256, 256, 64, 512
NCORES = 8
BL = B // NCORES  # 32 per-core batch
G4 = 4 * H  # 2048

_cache = {}


def _prep_weights(W_ih, W_hh, b_ih, b_hh, W_gh, b_gh, W_gx, b_gx,
                  W_hist, b_hist, W_feat, b_feat, W_comb, b_comb):
    """Host-side constant prep: permute/scale gate weights into the hybrid
    layout, build transposed chunks, masks, bias rows."""
    f32, bf16 = np.float32, ml_dtypes.bfloat16
    # hybrid gate position (j strip, g' in order i,f,o,g, ho) -> torch row
    base = {0: 0, 1: H, 2: 3 * H, 3: 2 * H}  # i,f,o,g -> torch i,f,g,o bases
    rows = np.zeros(G4, dtype=np.int64)
    scale = np.zeros(G4, dtype=np.float32)
    for j in range(4):
        for gp in range(4):
            for ho in range(128):
                pos = 512 * j + 128 * gp + ho
                rows[pos] = base[gp] + 128 * j + ho
                scale[pos] = 0.5 if gp == 0 else 1.0  # tanh-trick on i only
    Wih_p = (W_ih[rows] * scale[:, None]).astype(f32)   # [2048, 128]
    Whh_p = (W_hh[rows] * scale[:, None]).astype(f32)   # [2048, 512]
    bias_p = ((b_ih + b_hh)[rows] * scale).astype(f32)  # [2048]

    out = {}
    # gates h-chunk streams: Rh[j2] [128, 2048] = Whh_p[:, 128*j2+k].T
    for j2 in range(4):
        out[f"Rh{j2}"] = np.ascontiguousarray(
            Whh_p[:, 128 * j2:128 * (j2 + 1)].T).astype(bf16)
    out["Rcc"] = np.ascontiguousarray(Wih_p[:, :C].T).astype(bf16)  # [64,2048]
    Rm = np.zeros((C + 1, G4), dtype=f32)
    Rm[:C] = Wih_p[:, C:].T
    Rm[C] = bias_p
    out["Rm65"] = Rm.astype(bf16)  # [65, 2048]
    # gamma_h path-B chunks with bias(+ln2) row: [65, 128]
    for j2 in range(4):
        w = np.zeros((C + 1, 128), dtype=f32)
        w[:C] = W_gh[128 * j2:128 * (j2 + 1), :].T
        w[C] = b_gh[128 * j2:128 * (j2 + 1)]
        out[f"Wgh{j2}"] = w.astype(bf16)
    # x_h path-B chunks [128, 64] + bias row [1, 64]
    for j2 in range(4):
        out[f"Whist{j2}"] = np.ascontiguousarray(
            W_hist[:, 128 * j2:128 * (j2 + 1)].T).astype(bf16)
    out["bhist1"] = b_hist.reshape(1, C).astype(bf16)
    # z2: masked feat regression, no bias, bf16 [64, 64]
    Wfm = W_feat * (1.0 - np.eye(C, dtype=f32))
    out["WfT"] = np.ascontiguousarray(Wfm.T).astype(bf16)
    # zpre: [Wfm.T; b_feat] [65, 64]
    Wfv = np.zeros((C + 1, C), dtype=f32)
    Wfv[:C] = Wfm.T
    Wfv[C] = b_feat
    out["Wfv65"] = Wfv.astype(bf16)
    # alpha: two K-chunks. x-part [64, 64] bf16; m-part with bias row [65,64]
    out["WcombX"] = np.ascontiguousarray(W_comb[:, :C].T).astype(bf16)
    Wcm = np.zeros((C + 1, C), dtype=f32)
    Wcm[:C] = W_comb[:, C:].T
    Wcm[C] = b_comb
    out["WcombM65"] = Wcm.astype(bf16)
    # gamma_x per-partition scale/bias columns (fp32)
    out["wgx_neg"] = (-np.diag(W_gx)).reshape(C, 1).astype(f32)
    out["bgx_neg"] = (-b_gx).reshape(C, 1).astype(f32)
    out["ident"] = np.eye(128, dtype=f32)
    out["identb"] = np.eye(128, dtype=f32).astype(bf16)
    out["ones1"] = np.ones((1, BL), dtype=bf16)
    return out


def _build_nc(Tn):
    import concourse.bass as bass
    import concourse.bacc as bacc
    import concourse.mybir as mybir
    from concourse.tile import TileContext

    dt = mybir.dt
    AF = mybir.ActivationFunctionType
    ALU = mybir.AluOpType

    nc = bacc.Bacc(None, target_bir_lowering=False, debug=False)

    data_in = nc.declare_dram_parameter("data", [BL, Tn, C], dt.bfloat16, isOutput=False)
    out_d = nc.declare_dram_parameter("out", [BL, Tn, C], dt.bfloat16, isOutput=True)
    import os as _os
    DBG = _os.environ.get("DBG2") == "1"
    if DBG:
        dbg = {
            "d65a": nc.declare_dram_parameter("dbg_d65a", [C + 1, BL, Tn], dt.bfloat16, isOutput=True),
            "eg": nc.declare_dram_parameter("dbg_eg", [128, 4, BL, Tn], dt.bfloat16, isOutput=True),
            "xh": nc.declare_dram_parameter("dbg_xh", [C, BL], dt.float32, isOutput=True),
            "pg": nc.declare_dram_parameter("dbg_pg", [128, 512], dt.float32, isOutput=True),
            "hg": nc.declare_dram_parameter("dbg_hg", [128, 128], dt.float32, isOutput=True),
        }
    wspec = [
        ("Rh0", [128, G4], dt.bfloat16), ("Rh1", [128, G4], dt.bfloat16),
        ("Rh2", [128, G4], dt.bfloat16), ("Rh3", [128, G4], dt.bfloat16),
        ("Rcc", [C, G4], dt.bfloat16), ("Rm65", [C + 1, G4], dt.bfloat16),
        ("Wgh0", [C + 1, 128], dt.bfloat16), ("Wgh1", [C + 1, 128], dt.bfloat16),
        ("Wgh2", [C + 1, 128], dt.bfloat16), ("Wgh3", [C + 1, 128], dt.bfloat16),
        ("Whist0", [128, C], dt.bfloat16), ("Whist1", [128, C], dt.bfloat16),
        ("Whist2", [128, C], dt.bfloat16), ("Whist3", [128, C], dt.bfloat16),
        ("bhist1", [1, C], dt.bfloat16),
        ("WfT", [C, C], dt.bfloat16), ("Wfv65", [C + 1, C], dt.bfloat16),
        ("WcombX", [C, C], dt.bfloat16), ("WcombM65", [C + 1, C], dt.bfloat16),
        ("wgx_neg", [C, 1], dt.float32), ("bgx_neg", [C, 1], dt.float32),
        ("ident", [128, 128], dt.float32), ("identb", [128, 128], dt.bfloat16),
        ("ones1", [1, BL], dt.bfloat16),
    ]
    wdram = {n: nc.declare_dram_parameter(n, s, d, isOutput=False) for n, s, d in wspec}

    sb = {}
    for n, s, d in wspec:
        sb[n] = nc.alloc_sbuf_tensor(f"w_{n}", s, d)

    NF = BL * Tn  # flat (b, t) length
    CS = min(512, NF)  # bulk chunk size
    NCH = NF // CS

    # persistent stores (alive through the loop); free dims (b, t).
    # Bulk-phase transients are OVERLAID on persistents that are written
    # later (same tensor -> Tile tracks the reuse as region deps):
    #   a_sc -> Qst, r_sc -> M1, gamma_x -> ccs, v65 -> egam[:, j2=0] region.
    m65 = nc.alloc_sbuf_tensor("m65", [C + 1, BL, Tn], dt.bfloat16)
    Qst = nc.alloc_sbuf_tensor("Qst", [C, BL, Tn], dt.bfloat16)
    M1 = nc.alloc_sbuf_tensor("M1", [C, BL, Tn], dt.bfloat16)
    Rst = nc.alloc_sbuf_tensor("Rst", [C, BL, Tn], dt.bfloat16)
    ccs = nc.alloc_sbuf_tensor("ccs", [C, BL, Tn], dt.bfloat16)
    Cst = nc.alloc_sbuf_tensor("Cst", [128, 128], dt.float32)
    d65 = nc.alloc_sbuf_tensor("d65", [C + 1, BL, Tn], dt.bfloat16)
    egam = nc.alloc_sbuf_tensor("egam", [128, 4, BL, Tn], dt.bfloat16)
    TQ = min(16, Tn)
    dbm = nc.alloc_sbuf_tensor("dbm", [BL, TQ * C], dt.bfloat16)

    flat = "c b t -> c (b t)"

    with TileContext(nc) as tc:
        with (
            tc.tile_pool(name="ps_g", bufs=2, space="PSUM") as ps_g,
            tc.tile_pool(name="ps_s", bufs=2, space="PSUM") as ps_s,
            tc.tile_pool(name="ps_b", bufs=4, space="PSUM") as ps_b,
            tc.tile_pool(name="sb_loop", bufs=2) as sbl,
            tc.tile_pool(name="sb_stage", bufs=2) as sbs,
        ):
            for n, _, _ in wspec:
                nc.sync.dma_start(out=sb[n][:, :], in_=wdram[n][:, :])
            nc.vector.memset(m65[C:C + 1, :, :], 1.0)
            nc.vector.memset(d65[C:C + 1, :, :], 1.0)
            d65f = d65[:, :, :].rearrange("c b t -> c (b t)")

            # v65 overlays egam's j2=0 block: [65 rows, (b t)]
            v65 = egam[0:C + 1, 0:1, :, :]
            v65f = v65.rearrange("c j b t -> c (j b t)")
            v65c = v65f[0:C, :]
            nc.gpsimd.memset(v65f[:, :], 0.0)
            nc.vector.memset(v65f[C:C + 1, :], 1.0)

            # ---- load + transpose data into v/m stores
            for q in range(Tn // TQ):
                nc.sync.dma_start(
                    out=dbm[:, :],
                    in_=data_in[:, q * TQ:(q + 1) * TQ, :].rearrange(
                        "b t c -> b (t c)"))
                for g in range(TQ // 8):
                    pt = ps_b.tile([C, 8 * BL], dt.bfloat16, tag="bulk")
                    for k in range(8):
                        nc.tensor.transpose(
                            pt[:, k * BL:(k + 1) * BL],
                            dbm[:, (g * 8 + k) * C:(g * 8 + k + 1) * C],
                            sb["identb"][:BL, :BL])
                    t0 = q * TQ + g * 8
                    scr = sbs.tile([C, 8 * BL], dt.float32, tag="scr")
                    nc.vector.tensor_copy(scr[:, :], pt[:, :])
                    scb = sbs.tile([C, 8 * BL], dt.bfloat16, tag="scb")
                    nc.gpsimd.tensor_copy(scb[:, :], scr[:, :])
                    mu8 = sbs.tile([C, 8 * BL], dt.uint8, tag="mu8")
                    sv = scr[:, :].rearrange("c (k b) -> c k b", k=8)
                    svb = scb[:, :].rearrange("c (k b) -> c k b", k=8)
                    mvb = mu8[:, :].rearrange("c (k b) -> c k b", k=8)
                    m1 = m65[:C, :, t0:t0 + 8].rearrange("c b k -> c k b")
                    nc.vector.tensor_tensor(m1, sv, sv, ALU.is_equal)
                    nc.vector.tensor_tensor(mvb, sv, sv, ALU.is_equal)
                    dv = v65[0:C, :, :, t0:t0 + 8].rearrange(
                        "c j b k -> c (j k) b")
                    nc.vector.copy_predicated(dv, mvb, svb)

            # ---- delta scan: a = 1 - m shifted by one t (t>=2)
            # a_sc overlays Qst; r_sc overlays M1 (both rewritten later).
            a_sc, r_sc = Qst, M1
            nc.gpsimd.memset(r_sc[:, :, :], 1.0)
            nc.gpsimd.memset(r_sc[:, :, 0], 0.0)
            nc.gpsimd.memset(a_sc[:, :, :2], 0.0)
            nc.vector.tensor_scalar(a_sc[:, :, 2:], m65[:C, :, 1:Tn - 1],
                                    -1.0, 1.0, ALU.mult, ALU.add)
            nc.vector.tensor_tensor_scan(
                d65[:C, :, :].rearrange(flat),
                a_sc[:, :, :].rearrange(flat),
                r_sc[:, :, :].rearrange(flat),
                0.0, ALU.mult, ALU.add)
            if DBG:
                nc.sync.dma_start(out=dbg["d65a"][:, :, :], in_=d65[:, :, :])

            # ---- gamma_x (overlays ccs), alpha -> Q ; m1 ; zpre -> R
            gx_st = ccs
            gxf = gx_st[:, :, :].rearrange(flat)
            nc.scalar.activation(gxf, d65[:C, :, :].rearrange(flat),
                                 AF.Exp, bias=sb["bgx_neg"][:, 0:1],
                                 scale=sb["wgx_neg"][:, 0:1])
            nc.vector.tensor_scalar_min(gxf, gxf, 1.0)
            m1f = M1[:, :, :].rearrange(flat)
            nc.vector.tensor_scalar(m1f, m65[:C, :, :].rearrange(flat),
                                    -1.0, 1.0, ALU.mult, ALU.add)
            Qf = Qst[:, :, :].rearrange(flat)
            Rf = Rst[:, :, :].rearrange(flat)
            m65f = m65[:, :, :].rearrange("c b t -> c (b t)")
            for k in range(NCH):
                s = slice(k * CS, (k + 1) * CS)
                pa = ps_b.tile([C, CS], dt.float32, tag="bulk")
                nc.tensor.matmul(pa[:, :], sb["WcombX"][:, :],
                                 gxf[:, s], start=True, stop=False)
                nc.tensor.matmul(pa[:, :], sb["WcombM65"][:, :],
                                 m65f[:, s], start=False, stop=True)
                nc.vector.tensor_tensor(Qf[:, s], pa[:, :], m1f[:, s],
                                        ALU.mult)
                pz = ps_b.tile([C, CS], dt.float32, tag="bulk")
                nc.tensor.matmul(pz[:, :], sb["Wfv65"][:, :],
                                 v65f[:, s], start=True, stop=True)
                nc.vector.tensor_tensor(Rf[:, s], pz[:, :], Qf[:, s],
                                        ALU.mult)
                nc.vector.tensor_tensor(Rf[:, s], Rf[:, s], v65c[:, s],
                                        ALU.add)

            # ---- egam bulk: gamma_h for all t, FM layout [ho, (j2, b, t)].
            # j2=0 block overlays v65 (its last reader is the zpre matmul
            # above); write j2=0 LAST so the other blocks can start earlier.
            for j2 in (1, 2, 3, 0):
                egv = egam[:, j2:j2 + 1, :, :].rearrange(
                    "p j b t -> p (j b t)")
                for k in range(NCH):
                    s = slice(k * CS, (k + 1) * CS)
                    pe = ps_b.tile([128, CS], dt.float32, tag="bulk")
                    nc.tensor.matmul(pe[:, :], sb[f"Wgh{j2}"][:, :],
                                     d65f[:, s], start=True, stop=True)
                    nc.scalar.activation(egv[:, s], pe[:, :], AF.Exp,
                                         scale=-1.0)
                    nc.vector.tensor_scalar_min(egv[:, s], egv[:, s], 1.0)
            if DBG:
                nc.sync.dma_start(
                    out=dbg["eg"][:, :, :, :], in_=egam[:, :, :, :])

            # ---------------- recurrent loop ----------------
            hgam = sbl.tile([128, 128], dt.bfloat16, tag="hgam")
            nc.vector.memset(hgam[:, :], 0.0)
            nc.vector.memset(Cst[:, :], 0.0)
            for t in range(Tn):
                last = (t == Tn - 1)
                # ---- gates PSUM: m65 group (earliest, no h dependence)
                if not last:
                    pg = ps_g.tile([128, 512], dt.float32, tag="pg")
                    for j in range(4):
                        nc.tensor.matmul(pg[32 * j:32 * (j + 1), :],
                                         m65[:, :, t],
                                         sb["Rm65"][:, 512 * j:512 * (j + 1)],
                                         start=True, stop=False,
                                         tile_position=(0, 32 * j))
                # ---- xh = Whist @ hgam + b (feature-major [64, 32])
                ps = ps_s.tile([128, 512], dt.float32, tag="ps")
                xh_ps = ps[0:C, 0:BL]
                z2_ps = ps[0:C, BL:2 * BL]
                to_ps = ps[:, 128:256]
                tc_ps = ps[:, 256:384]
                for j2 in range(4):
                    nc.tensor.matmul(xh_ps, sb[f"Whist{j2}"][:, :],
                                     hgam[:, j2 * BL:(j2 + 1) * BL],
                                     start=(j2 == 0), stop=False)
                nc.tensor.matmul(xh_ps, sb["bhist1"][:, :], sb["ones1"][:, :],
                                 start=False, stop=True)
                # ---- h-part gates, chunks 0-1
                if not last:
                    for j2 in (0, 1):
                        for j in range(4):
                            nc.tensor.matmul(pg[32 * j:32 * (j + 1), :],
                                             hgam[:, j2 * BL:(j2 + 1) * BL],
                                             sb[f"Rh{j2}"][:, 512 * j:512 * (j + 1)],
                                             start=False, stop=False,
                                             tile_position=(0, 32 * j))
                # ---- imputation chain
                xhp = sbl.tile([C, BL], dt.bfloat16, tag="xhp")
                nc.vector.tensor_tensor(xhp[:, :], xh_ps, M1[:, :, t], ALU.mult)
                nc.tensor.matmul(z2_ps, sb["WfT"][:, :], xhp[:, :],
                                 start=True, stop=True)
                # ---- h-part gates, chunks 2-3
                if not last:
                    for j2 in (2, 3):
                        for j in range(4):
                            nc.tensor.matmul(pg[32 * j:32 * (j + 1), :],
                                             hgam[:, j2 * BL:(j2 + 1) * BL],
                                             sb[f"Rh{j2}"][:, 512 * j:512 * (j + 1)],
                                             start=False, stop=False,
                                             tile_position=(0, 32 * j))
                w2 = sbl.tile([C, BL], dt.float32, tag="w2")
                nc.vector.tensor_tensor(w2[:, :], xh_ps, Qst[:, :, t], ALU.mult)
                rx = sbl.tile([C, BL], dt.float32, tag="rx")
                nc.vector.tensor_tensor(rx[:, :], xhp[:, :], Rst[:, :, t], ALU.add)
                rx2 = sbl.tile([C, BL], dt.float32, tag="rx2")
                nc.vector.tensor_tensor(rx2[:, :], rx[:, :], w2[:, :], ALU.subtract)
                w1 = sbl.tile([C, BL], dt.float32, tag="w1")
                nc.vector.tensor_tensor(w1[:, :], z2_ps, Qst[:, :, t], ALU.mult)
                # cc -> ccs[:, :, t]  (output value, bf16)
                nc.vector.tensor_tensor(ccs[:, :, t], w1[:, :], rx2[:, :], ALU.add)
                if DBG and t == 0:
                    dxh = sbl.tile([C, BL], dt.float32, tag="dbgxh")
                    nc.vector.tensor_copy(dxh[:, :], xh_ps)
                    nc.sync.dma_start(out=dbg["xh"][:, :], in_=dxh[:, :])
                if last:
                    continue
                # ---- cc-part gates (final accumulation group)
                for j in range(4):
                    nc.tensor.matmul(pg[32 * j:32 * (j + 1), :], ccs[:, :, t],
                                     sb["Rcc"][:, 512 * j:512 * (j + 1)],
                                     start=False, stop=True,
                                     tile_position=(0, 32 * j))
                # ---- LSTM pointwise (Cst = 2c; sigmoid for f/o, tanh-trick i)
                tg = sbl.tile([128, 384], dt.bfloat16, tag="tg")  # i, f, g
                tgo = sbl.tile([128, 128], dt.float32, tag="tgo")  # sig(o)
                nc.scalar.activation(tg[:, 128:256], pg[:, 128:256], AF.Sigmoid)
                nc.scalar.activation(tg[:, 0:128], pg[:, 0:128], AF.Tanh)      # i
                nc.scalar.activation(tg[:, 256:384], pg[:, 384:512], AF.Tanh)  # g
                nc.scalar.activation(tgo[:, :], pg[:, 256:384], AF.Sigmoid)    # o
                Bt = sbl.tile([128, 128], dt.float32, tag="Bt")
                nc.vector.scalar_tensor_tensor(Bt[:, :], tg[:, 0:128], 1.0,
                                               tg[:, 256:384], ALU.add, ALU.mult)
                A1 = sbl.tile([128, 128], dt.float32, tag="A1")
                nc.vector.tensor_tensor(A1[:, :], tg[:, 128:256], Cst[:, :],
                                        ALU.mult)
                nc.vector.tensor_tensor(Cst[:, :], A1[:, :], Bt[:, :], ALU.add)
                # transposes: sig(o) and new Cst (both fp32) -> FM
                nc.tensor.transpose(to_ps, tgo[:, :], sb["ident"][:, :])
                nc.tensor.transpose(tc_ps, Cst[:, :], sb["ident"][:, :])
                eo = sbl.tile([128, 128], dt.bfloat16, tag="eo")
                nc.vector.tensor_tensor(
                    eo[:, :], to_ps,
                    egam[:, :, :, t + 1:t + 2].rearrange("p j b t -> p (j b t)"),
                    ALU.mult)
                tcn = sbl.tile([128, 128], dt.bfloat16, tag="tcn")
                nc.scalar.activation(tcn[:, :], tc_ps, AF.Tanh, scale=0.5)
                hgam = sbl.tile([128, 128], dt.bfloat16, tag="hgam")
                nc.vector.tensor_tensor(hgam[:, :], eo[:, :], tcn[:, :], ALU.mult)
                if DBG and t == 0:
                    dpg = sbl.tile([128, 512], dt.float32, tag="dbgpg")
                    nc.vector.tensor_copy(dpg[:, :], pg[:, :])
                    nc.sync.dma_start(out=dbg["pg"][:, :], in_=dpg[:, :])
                    dscr = sbl.tile([128, 128], dt.float32, tag="dbghg")
                    nc.vector.tensor_copy(dscr[:, :], hgam[:, :])
                    nc.sync.dma_start(out=dbg["hg"][:, :], in_=dscr[:, :])

            # ---------------- output: bulk transpose + DMA ----------------
            ccf = ccs[:, :, :].rearrange(flat)
            outf = out_d[:, :, :].rearrange("b t c -> (b t) c")
            NBLK = NF // 128
            GRP = min(4, NBLK)
            for g0 in range(0, NBLK, GRP):
                po = ps_b.tile([128, GRP * C], dt.bfloat16, tag="bulk")
                for k in range(GRP):
                    blk = g0 + k
                    nc.tensor.transpose(po[:, k * C:(k + 1) * C],
                                        ccf[:, 128 * blk:128 * (blk + 1)],
                                        sb["identb"][:C, :C])
                stg = sbs.tile([128, GRP * C], dt.bfloat16, tag="stg")
                nc.vector.tensor_copy(stg[:, :], po[:, :])
                nc.sync.dma_start(
                    out=outf[128 * g0:128 * (g0 + GRP), :].rearrange(
                        "(k p) c -> p k c", k=GRP),
                    in_=stg[:, :].rearrange("p (k c) -> p k c", k=GRP))
    nc.compile()
    return nc


def _build_exec(nc):
    """Build a cached jitted executor for nc (clone of bass2jax.run_bass_via_pjrt
    minus donation, so device-resident inputs can be reused across calls)."""
    import jax
    from jax.experimental.shard_map import shard_map
    from jax.sharding import Mesh, PartitionSpec
    from concourse import bass2jax as b2j
    import concourse.mybir as mybir
    b2j.install_neuronx_cc_hook()

    partition_name = nc.partition_id_tensor.name if nc.partition_id_tensor else None
    in_names, out_names, out_avals, zero_outs = [], [], [], []
    for alloc in nc.m.functions[0].allocations:
        if not isinstance(alloc, mybir.MemoryLocationSet):
            continue
        name = alloc.memorylocations[0].name
        if alloc.kind == "ExternalInput":
            if name != partition_name:
                in_names.append(name)
        elif alloc.kind == "ExternalOutput":
            out_names.append(name)
            shape = tuple(alloc.tensor_shape)
            dtype = mybir.dt.np(alloc.dtype)
            out_avals.append(jax.core.ShapedArray(shape, dtype))
            zero_outs.append(np.zeros(shape, dtype))
    n_params = len(in_names)
    bind_in_names = list(in_names) + list(out_names)
    if partition_name is not None:
        bind_in_names.append(partition_name)

    def _body(*args):
        operands = list(args)
        if partition_name is not None:
            operands.append(b2j.partition_id_tensor())
        outs = b2j._bass_exec_p.bind(
            *operands,
            out_avals=tuple(out_avals),
            in_names=tuple(bind_in_names),
            out_names=tuple(out_names),
            lowering_input_output_aliases=(),
            sim_require_finite=True,
            sim_require_nnan=True,
            nc=nc,
        )
        return tuple(outs)

    devices = jax.devices()[:NCORES]
    mesh = Mesh(np.asarray(devices), ("core",))
    in_specs = (PartitionSpec("core"),) * (n_params + len(out_names))
    out_specs = (PartitionSpec("core"),) * len(out_names)
    fn = jax.jit(
        shard_map(_body, mesh=mesh, in_specs=in_specs, out_specs=out_specs,
                  check_rep=False),
        keep_unused=True)
    from jax.sharding import NamedSharding
    shard = NamedSharding(mesh, PartitionSpec("core"))
    zdev = [
        jax.device_put(np.zeros((NCORES * z.shape[0], *z.shape[1:]), z.dtype),
                       shard)
        for z in zero_outs
    ]
    return dict(fn=fn, in_names=in_names, out_names=out_names, zdev=zdev,
                shard=shard, wdev=None, wfp=None)


_last_results = None
_cache_nc = {}

_WKEYS = ("W_ih", "W_hh", "b_ih", "b_hh", "W_gh", "b_gh", "W_gx", "b_gx",
          "W_hist", "b_hist", "W_feat", "b_feat", "W_comb", "b_comb")


def kernel(**inputs):
    import jax
    import hashlib
    data = np.asarray(inputs["data"], dtype=np.float32)
    Tn = data.shape[1]
    if Tn not in _cache:
        nc = _build_nc(Tn)
        _cache[Tn] = _build_exec(nc)
        _cache_nc[Tn] = nc
    ent = _cache[Tn]

    h = hashlib.blake2b(digest_size=16)
    for k in _WKEYS:
        h.update(np.ascontiguousarray(np.asarray(inputs[k], np.float32)).tobytes())
    wfp = h.digest()
    if ent["wfp"] != wfp:
        prep = _prep_weights(*[inputs[k] for k in _WKEYS])
        prep = {k: np.ascontiguousarray(v) for k, v in prep.items()}
        wdev = {}
        for name in ent["in_names"]:
            if name == "data":
                continue
            arr = np.concatenate([prep[name]] * NCORES, axis=0)
            wdev[name] = jax.device_put(arr, ent["shard"])
        ent["wdev"] = wdev
        ent["wfp"] = wfp

    ddev = jax.device_put(
        np.ascontiguousarray(data.astype(ml_dtypes.bfloat16)), ent["shard"])
    args = [ddev if name == "data" else ent["wdev"][name]
            for name in ent["in_names"]]
    outs = ent["fn"](*args, *ent["zdev"])
    res = {name: outs[i] for i, name in enumerate(ent["out_names"])}
    globals()["_last_results"] = res
    return np.asarray(res["out"]).astype(np.float32)


if __name__ == "__main__":
    import reference
    inp = reference.setup_inputs()
    inp = {k: np.asarray(v) for k, v in inp.items()}
    Tn = int(os.environ.get("TN", "8"))
    inp["data"] = inp["data"][:, :Tn]
    exp = np.asarray(reference.reference(**{k: v for k, v in inp.items()}))
    act = kernel(**inp)
    err = np.abs(act - exp)
    rel = np.linalg.norm((act - exp).ravel()) / np.linalg.norm(exp.ravel())
    print("max abs err:", np.nanmax(err), "rel:", rel)

